# revision 31
# baseline (speedup 1.0000x reference)
"""CliffordLinear (Cl(3,0)) Trainium2 kernel — fp8 DoubleRow + Gauss edition.

Math: Cl(3,0) ~= 2x2 complex matrices via Pauli rep phi.  The reference's
per-channel Clifford contraction maps to OutM[b,o] = sum_i phi(W[o,i]) @
phi(X[b,i]).  Per output column c in {0,1}, with A = phi(W) as 512x512
[(i,m) x (o,r)] planes Ar/Ai and x column halves xr/xi, the 3-multiply
(Karatsuba/Gauss) form shares one product between Re and Im:

    k1 = Ar @ (xr + xi)      k2' = -(Ar+Ai) @ xi      k3 = (Ai-Ar) @ xr
    Re = k1 + k2'            Im = k1 + k3

k2' and k3 accumulate in PSUM; k1 is evicted once (ScalarE) and added to
both via one broadcast (stride-0) DVE op per column half -- 3/4 the PE
work of the plain 4-multiply form at the cost of 1.5x x traffic.

Precision/performance: all matmul operands are fp8e4 (e4m3) hi+lo pairs
prepared on the host (v ~= hi + lo, ~13 mantissa bits); each product runs
three fp8 DoubleRow passes (hi*hi + lo*hi + hi*lo; the dropped lo*lo term
is ~1e-3 relative), contracting 256 rows per matmul at 0.5 cycles/row --
2x the bf16/f32r column rate, so Gauss x 3 passes = 9/16 of the baseline
f32r PE time.  Inputs stream as fp8 pairs, outputs as bf16 (host rescales
to f32).  The inverse-Pauli butterfly runs on packed bf16 APs (DVE 2x_1p
mode for adds, Pool for subs), and the last b-tile computes c1 before c0
and drains in column halves so the tail overlaps the final matmuls.

Sharding: data-parallel over batch (1024 rows/core); weights replicated.
Per-core HBM traffic: 6.3 MB x + 1.5 MB w in, 4.2 MB out.
"""

import sys

sys.path.insert(0, "/opt/trn_rl_repo")

import numpy as np
import ml_dtypes

import concourse.bass as bass  # noqa: F401  (registers lowerings)
import concourse.mybir as mybir
import concourse.tile as tile
from concourse import bacc
from concourse.bass_utils import run_bass_kernel_spmd

N_CORES = 8
B, CIN, COUT, NB = 8192, 256, 256, 8
BS = B // N_CORES          # 1024 batch rows per core
K = CIN * 4                # 1024 contraction rows (re|im halves)
HK = K // 2                # 512 rows per half (one Gauss operand)
OUTW = COUT * NB           # 2048 output width
BT = BS // 128             # 8 b-tiles
E4 = ml_dtypes.float8_e4m3
SX = 16.0                  # x scale before e4m3 quantization
SW = 1024.0                # w scale before e4m3 quantization

_cached = {}


def _apn(base, off, dims):
    """AP with explicit free dims [(step, n), ...] at column offset off."""
    a = base.copy()
    part = a.ap.to_list()[0]
    v = a.ap
    v.clear()
    v.extend([tuple(part)] + [tuple(d) for d in dims])
    a.offset = a.offset + off
    return a


def _build_nc():
    f32 = mybir.dt.float32
    bf16 = mybir.dt.bfloat16
    fp8 = mybir.dt.float8e4
    DR = mybir.MatmulPerfMode.DoubleRow
    nc = bacc.Bacc("TRN2", target_bir_lowering=False, debug=False,
                   num_devices=N_CORES)
    # x'[c]: [bt, p, hl, op, kt, b] flat [bt, 128, 3072] fp8;
    # op 0/1/2 = xs (xr+xi) / xi / xr; row kappa = kt*128 + p (4 kt per op).
    xt0 = nc.dram_tensor("xt0", [BT, 128, 3072], fp8, kind="ExternalInput")
    xt1 = nc.dram_tensor("xt1", [BT, 128, 3072], fp8, kind="ExternalInput")
    # w kpair blocks [P1_01, P1_23, P2_01, P2_23, P3_01, P3_23] for planes
    # P1 = Ar, P2 = -(Ar+Ai), P3 = Ai-Ar; each block [p, t, c].
    w8 = nc.dram_tensor("w8", [2, 128, 6144], fp8, kind="ExternalInput")
    # out cols: h*1024 + l*128 + o_loc  (h = o-half, l = blade)
    out = nc.dram_tensor("out", [BS, OUTW], bf16, kind="ExternalOutput")

    with tile.TileContext(nc) as tc:
        with tc.tile_pool(name="wpool", bufs=1) as wpool, \
             tc.tile_pool(name="xpool", bufs=3) as xpool, \
             tc.tile_pool(name="opool", bufs=3) as opool, \
             tc.tile_pool(name="pspool", bufs=2, space="PSUM") as pspool:
            # PE warmup: ramp the clock gate during the initial DMA wait.
            warm_in = wpool.tile([128, 640], bf16, tag="warm_in")
            nc.vector.memset(warm_in[:], 0.0)
            warm_ps = pspool.tile([128, 512], f32, tag="pk0")
            for _ in range(8):
                nc.tensor.matmul(warm_ps[:], warm_in[:, :128], warm_in[:, 128:640],
                                 start=True, stop=True)

            wh = wpool.tile([128, 6144], fp8, tag="wh")
            wl = wpool.tile([128, 6144], fp8, tag="wl")
            x0_pre = xpool.tile([128, 2, 3, 4, 128], fp8, tag="x0")
            x1_pre = xpool.tile([128, 2, 3, 4, 128], fp8, tag="x1")
            def wdma(t, hl, blk):
                nc.sync.dma_start(t[:, blk * 1024:(blk + 2) * 1024],
                                  w8[hl][:, blk * 1024:(blk + 2) * 1024])
            nc.sync.dma_start(x0_pre[:, 0], xt0[0][:, 0:1536])      # x0 hi
            wdma(wh, 0, 0)                                          # P1 hi
            nc.sync.dma_start(x0_pre[:, 1], xt0[0][:, 1536:3072])   # x0 lo
            nc.sync.dma_start(x1_pre[:, 0], xt1[0][:, 0:1536])      # x1 hi
            wdma(wl, 1, 0)                                          # P1 lo
            nc.sync.dma_start(x1_pre[:, 1], xt1[0][:, 1536:3072])   # x1 lo
            wdma(wh, 0, 2)                                          # P2 hi
            wdma(wl, 1, 2)                                          # P2 lo
            wdma(wh, 0, 4)                                          # P3 hi
            wdma(wl, 1, 4)                                          # P3 lo

            def half_groups(xs, pk, h):
                """k1/k2'/k3 accumulation groups for one (c, o-half):
                pk cols [g*256, g*256+256), w cols [256h, 256h+256)."""
                wc0 = 256 * h
                for g in range(3):
                    col0 = 256 * g
                    for pi, w_t in enumerate((wh, wh, wl)):
                        hlx = (0, 1, 0)[pi]
                        for j in range(2):
                            nc.tensor.matmul(
                                pk[:, col0:col0 + 256],
                                xs[:, hlx, g, 2 * j:2 * j + 2, :],
                                _apn(w_t[:], (2 * g + j) * 1024 + wc0,
                                     [(512, 2), (1, 256)]),
                                start=(pi == 0 and j == 0),
                                stop=(pi == 2 and j == 1),
                                perf_mode=DR)

            def evict_k1(pk, tag):
                """k1 (pk cols [0,256)) -> SBUF bf16 [r, o] (col r*128+o)."""
                s = opool.tile([128, 256], bf16, tag=tag)
                nc.scalar.copy(
                    _apn(s[:], 0, [(1, 128), (128, 2)]),
                    _apn(pk[:], 0, [(2, 128), (1, 2)]))
                return s

            def recomb(pk, sk1, tag):
                """comps [j, r, o] bf16 (col j*256 + r*128 + o):
                Re = k2' + k1, Im = k3 + k1 in one broadcast dual op."""
                comps = opool.tile([128, 512], bf16, tag=tag)
                nc.vector.tensor_add(
                    _apn(comps[:], 0, [(256, 2), (1, 128), (128, 2)]),
                    _apn(pk[:], 256, [(256, 2), (2, 128), (1, 2)]),
                    _apn(sk1[:], 0, [(0, 2), (1, 128), (128, 2)]))
                return comps

            def bfly(c0, c1, h, stage, tail=False):
                """8 blades for o-half h; stage cols 1024h + l*128 + o_loc.
                Adds on DVE (2x packed bf16); subs on Pool mid-run, DVE on
                the tail."""
                base = 1024 * h
                o2 = [(1, 128)]
                sub2 = nc.vector.tensor_sub if tail else nc.gpsimd.tensor_sub
                nc.vector.tensor_add(                      # x0, x7
                    _apn(stage[:], base + 0 * 128, [(896, 2)] + o2),
                    _apn(c0[:], 0, [(256, 2)] + o2),
                    _apn(c1[:], 128, [(256, 2)] + o2))
                sub2(                                      # x4, x3
                    _apn(stage[:], base + 4 * 128, [(-128, 2)] + o2),
                    _apn(c0[:], 0, [(256, 2)] + o2),
                    _apn(c1[:], 128, [(256, 2)] + o2))
                nc.vector.tensor_add(                      # x1, x6
                    _apn(stage[:], base + 1 * 128, [(640, 2)] + o2),
                    _apn(c0[:], 128, [(256, 2)] + o2),
                    _apn(c1[:], 0, [(256, 2)] + o2))
                sub2(                                      # x5, x2
                    _apn(stage[:], base + 5 * 128, [(-384, 2)] + o2),
                    _apn(c0[:], 128, [(256, 2)] + o2),
                    _apn(c1[:], 0, [(256, 2)] + o2))

            def process_half(xs, h, ctag):
                """matmuls + k1 eviction + recombination for one (c, half)."""
                pk = pspool.tile([128, 1024], f32, tag=f"pk{ctag}")
                half_groups(xs, pk, h)
                sk1 = evict_k1(pk, f"sk{ctag}{h}")
                return recomb(pk, sk1, f"c{ctag}{h}")

            for bt in range(BT):
                last = bt == BT - 1
                if bt == 0:
                    x0_s, x1_s = x0_pre, x1_pre
                else:
                    x0_s = xpool.tile([128, 2, 3, 4, 128], fp8, tag="x0")
                    x1_s = xpool.tile([128, 2, 3, 4, 128], fp8, tag="x1")
                    nc.sync.dma_start(x0_s[:], xt0[bt])
                    nc.sync.dma_start(x1_s[:], xt1[bt])
                stage = opool.tile([128, OUTW], bf16, tag="stage")
                row = out[bt * 128:(bt + 1) * 128, :]
                if not last:
                    c0a = process_half(x0_s, 0, 0)
                    c0b = process_half(x0_s, 1, 0)
                    c1a = process_half(x1_s, 0, 1)
                    bfly(c0a, c1a, 0, stage)
                    c1b = process_half(x1_s, 1, 1)
                    bfly(c0b, c1b, 1, stage)
                    nc.sync.dma_start(row, stage[:])
                else:
                    # tail: c1 first, drain in column halves.
                    c1a = process_half(x1_s, 0, 1)
                    c1b = process_half(x1_s, 1, 1)
                    c0a = process_half(x0_s, 0, 0)
                    bfly(c0a, c1a, 0, stage, tail=True)
                    nc.sync.dma_start(row[:, 0:1024], stage[:, 0:1024])
                    c0b = process_half(x0_s, 1, 0)
                    bfly(c0b, c1b, 1, stage, tail=True)
                    nc.sync.dma_start(row[:, 1024:2048], stage[:, 1024:2048])
    nc.finalize()
    return nc


def _pauli_parts(v):
    """v[..., 8] -> c0, c1 of shape [..., 2(m), 2(reim)]: the c-th column
    (Re, Im) of phi(v) rows m."""
    c0 = np.empty(v.shape[:-1] + (2, 2), dtype=v.dtype)
    c1 = np.empty_like(c0)
    v0, v1, v2, v3, v4, v5, v6, v7 = (v[..., a] for a in range(8))
    c0[..., 0, 0] = v0 + v4   # Re A
    c0[..., 0, 1] = v3 + v7   # Im A
    c0[..., 1, 0] = v1 + v5   # Re C
    c0[..., 1, 1] = v6 + v2   # Im C
    c1[..., 0, 0] = v1 - v5   # Re B
    c1[..., 0, 1] = v6 - v2   # Im B
    c1[..., 1, 0] = v0 - v4   # Re D
    c1[..., 1, 1] = v7 - v3   # Im D
    return c0, c1


def _hi_lo(v):
    """f32 array -> (hi, lo) e4m3 planes with hi + lo ~= v."""
    hi = v.astype(E4)
    lo = (v - hi.astype(np.float32)).astype(E4)
    return hi, lo


def _prep_w(weight):
    """weight [COUT, CIN, 8] -> [2, 128, 6144] fp8 kpair blocks of the
    Gauss planes [Ar, -(Ar+Ai), Ai-Ar], each block [128 p, 2 t, 512 c]."""
    w = weight.astype(np.float32)
    cw0, cw1 = _pauli_parts(w)
    R = np.empty((CIN, 2, COUT, 2), np.float32)   # [(i,m),(o,r)]
    I = np.empty_like(R)
    for m, cm in ((0, cw0), (1, cw1)):
        for r in range(2):
            R[:, m, :, r] = 0.5 * cm[:, :, r, 0].T
            I[:, m, :, r] = 0.5 * cm[:, :, r, 1].T
    R = R.reshape(HK, HK) * SW
    I = I.reshape(HK, HK) * SW
    out = np.empty((2, 128, 6144), dtype=E4)
    for hl in (0, 1):
        blocks = []
        for M in (R, -(R + I), I - R):
            h, l = _hi_lo(M)
            P = (h if hl == 0 else l).astype(np.float32)
            for j in (0, 1):
                blk = P[256 * j:256 * j + 256].reshape(2, 128, HK)
                blocks.append(blk.transpose(1, 0, 2))   # [128, 2, 512]
        arr = np.stack(blocks, axis=1)    # [128, 6, 2, 512]
        out[hl] = arr.reshape(128, 6144).astype(E4)
    return out


def _prep_x(x):
    """x [B, CIN, 8] -> two per-core arrays [N_CORES, BT, 128, 3072] fp8
    (c = 0, 1): ops [xs, xi, xr] in [bt, p, hl, op, kt, b] layout."""
    xf = x.astype(np.float32)
    c0, c1 = _pauli_parts(xf)          # [B, CIN, m, reim]
    outs = []
    for arr in (c0, c1):
        kb = arr.transpose(3, 1, 2, 0).reshape(K, B) * SX   # [K, B]
        xr, xi = kb[0:HK], kb[HK:K]
        ops = np.stack([xr + xi, xi, xr], axis=0)   # [op, 512, B]
        a = ops.reshape(3, 4, 128, N_CORES, BT, 128)  # [op, kt, p, core, bt, b]
        a = np.ascontiguousarray(a.transpose(3, 4, 2, 0, 1, 5))
        hi, lo = _hi_lo(a)                       # [core, bt, p, op, kt, b]
        packed = np.stack([hi, lo], axis=3)      # [core, bt, p, hl, op, kt, b]
        outs.append(np.ascontiguousarray(
            packed.reshape(N_CORES, BT, 128, 3072)))
    return outs


def kernel(x, weight, bias, cayley):
    assert x.shape == (B, CIN, NB) and weight.shape == (COUT, CIN, NB)
    if "nc" not in _cached:
        _cached["nc"] = _build_nc()
    nc = _cached["nc"]

    xt0, xt1 = _prep_x(np.asarray(x))
    w8 = _prep_w(np.asarray(weight))
    in_maps = [{"xt0": xt0[c], "xt1": xt1[c], "w8": w8} for c in range(N_CORES)]
    res = run_bass_kernel_spmd(nc, in_maps, core_ids=list(range(N_CORES)))
    out = np.concatenate(
        [res.results[c]["out"].astype(np.float32) for c in range(N_CORES)],
        axis=0)
    # device cols: (h, l, o_loc) -> out[b, o, l] with o = h*128 + o_loc
    out = out.reshape(B, 2, NB, 128).transpose(0, 1, 3, 2).reshape(B, COUT, NB)
    out = out * (1.0 / (SX * SW))
    out = out + np.asarray(bias, np.float32)[None]
    return out.astype(np.float32)


# revision 32
# speedup vs baseline: 1.0044x; 1.0044x over previous
"""CliffordLinear (Cl(3,0)) Trainium2 kernel — fp8 DoubleRow + Gauss edition.

Math: Cl(3,0) ~= 2x2 complex matrices via Pauli rep phi.  The reference's
per-channel Clifford contraction maps to OutM[b,o] = sum_i phi(W[o,i]) @
phi(X[b,i]).  Per output column c in {0,1}, with A = phi(W) as 512x512
[(i,m) x (o,r)] planes Ar/Ai and x column halves xr/xi, the 3-multiply
(Karatsuba/Gauss) form shares one product between Re and Im:

    k1 = Ar @ (xr + xi)      k2' = -(Ar+Ai) @ xi      k3 = (Ai-Ar) @ xr
    Re = k1 + k2'            Im = k1 + k3

k2' and k3 accumulate in PSUM; k1 is evicted once (ScalarE) and added to
both via one broadcast (stride-0) DVE op per column half -- 3/4 the PE
work of the plain 4-multiply form at the cost of 1.5x x traffic.

Precision/performance: all matmul operands are fp8e4 (e4m3) hi+lo pairs
prepared on the host (v ~= hi + lo, ~13 mantissa bits); each product runs
three fp8 DoubleRow passes (hi*hi + lo*hi + hi*lo; the dropped lo*lo term
is ~1e-3 relative), contracting 256 rows per matmul at 0.5 cycles/row --
2x the bf16/f32r column rate, so Gauss x 3 passes = 9/16 of the baseline
f32r PE time.  Inputs stream as fp8 pairs, outputs as bf16 (host rescales
to f32).  The inverse-Pauli butterfly runs on packed bf16 APs (DVE 2x_1p
mode for adds, Pool for subs), and the last b-tile computes c1 before c0
and drains in column halves so the tail overlaps the final matmuls.

Sharding: data-parallel over batch (1024 rows/core); weights replicated.
Per-core HBM traffic: 6.3 MB x + 1.5 MB w in, 4.2 MB out.
"""

import sys

sys.path.insert(0, "/opt/trn_rl_repo")

import numpy as np
import ml_dtypes

import concourse.bass as bass  # noqa: F401  (registers lowerings)
import concourse.mybir as mybir
import concourse.tile as tile
from concourse import bacc
from concourse.bass_utils import run_bass_kernel_spmd

N_CORES = 8
B, CIN, COUT, NB = 8192, 256, 256, 8
BS = B // N_CORES          # 1024 batch rows per core
K = CIN * 4                # 1024 contraction rows (re|im halves)
HK = K // 2                # 512 rows per half (one Gauss operand)
OUTW = COUT * NB           # 2048 output width
BT = BS // 128             # 8 b-tiles
E4 = ml_dtypes.float8_e4m3
SX = 16.0                  # x scale before e4m3 quantization
SW = 1024.0                # w scale before e4m3 quantization

_cached = {}


def _apn(base, off, dims):
    """AP with explicit free dims [(step, n), ...] at column offset off."""
    a = base.copy()
    part = a.ap.to_list()[0]
    v = a.ap
    v.clear()
    v.extend([tuple(part)] + [tuple(d) for d in dims])
    a.offset = a.offset + off
    return a


def _build_nc():
    f32 = mybir.dt.float32
    bf16 = mybir.dt.bfloat16
    fp8 = mybir.dt.float8e4
    DR = mybir.MatmulPerfMode.DoubleRow
    nc = bacc.Bacc("TRN2", target_bir_lowering=False, debug=False,
                   num_devices=N_CORES)
    # x'[c]: [bt, p, hl, op, kt, b] flat [bt, 128, 3072] fp8;
    # op 0/1/2 = xs (xr+xi) / xi / xr; row kappa = kt*128 + p (4 kt per op).
    xt0 = nc.dram_tensor("xt0", [BT, 128, 3072], fp8, kind="ExternalInput")
    xt1 = nc.dram_tensor("xt1", [BT, 128, 3072], fp8, kind="ExternalInput")
    # w kpair blocks [P1_01, P1_23, P2_01, P2_23, P3_01, P3_23] for planes
    # P1 = Ar, P2 = -(Ar+Ai), P3 = Ai-Ar; each block [p, t, c].
    w8 = nc.dram_tensor("w8", [2, 128, 6144], fp8, kind="ExternalInput")
    # out cols: h*1024 + l*128 + o_loc  (h = o-half, l = blade)
    out = nc.dram_tensor("out", [BS, OUTW], bf16, kind="ExternalOutput")

    with tile.TileContext(nc) as tc:
        with tc.tile_pool(name="wpool", bufs=1) as wpool, \
             tc.tile_pool(name="xpool", bufs=3) as xpool, \
             tc.tile_pool(name="opool", bufs=3) as opool, \
             tc.tile_pool(name="pspool", bufs=2, space="PSUM") as pspool:
            # PE warmup: ramp the clock gate during the initial DMA wait.
            warm_in = wpool.tile([128, 640], bf16, tag="warm_in")
            nc.vector.memset(warm_in[:], 0.0)
            warm_ps = pspool.tile([128, 512], f32, tag="pk0")
            for _ in range(8):
                nc.tensor.matmul(warm_ps[:], warm_in[:, :128], warm_in[:, 128:640],
                                 start=True, stop=True)

            wh = wpool.tile([128, 6144], fp8, tag="wh")
            wl = wpool.tile([128, 6144], fp8, tag="wl")
            x0_pre = xpool.tile([128, 2, 3, 4, 128], fp8, tag="x0")
            x1_pre = xpool.tile([128, 2, 3, 4, 128], fp8, tag="x1")
            nc.sync.dma_start(x0_pre[:, 0], xt0[0][:, 0:1536])      # x0 hi
            for blk in range(0, 6, 2):
                nc.sync.dma_start(wh[:, blk * 1024:(blk + 2) * 1024],
                                  w8[0][:, blk * 1024:(blk + 2) * 1024])
            nc.sync.dma_start(x0_pre[:, 1], xt0[0][:, 1536:3072])   # x0 lo
            for blk in range(0, 6, 2):
                nc.sync.dma_start(wl[:, blk * 1024:(blk + 2) * 1024],
                                  w8[1][:, blk * 1024:(blk + 2) * 1024])
            nc.sync.dma_start(x1_pre[:], xt1[0])

            def half_groups(xs, pk, h):
                """k1/k2'/k3 accumulation groups for one (c, o-half):
                pk cols [g*256, g*256+256), w cols [256h, 256h+256)."""
                wc0 = 256 * h
                for g in range(3):
                    col0 = 256 * g
                    for pi, w_t in enumerate((wh, wh, wl)):
                        hlx = (0, 1, 0)[pi]
                        for j in range(2):
                            nc.tensor.matmul(
                                pk[:, col0:col0 + 256],
                                xs[:, hlx, g, 2 * j:2 * j + 2, :],
                                _apn(w_t[:], (2 * g + j) * 1024 + wc0,
                                     [(512, 2), (1, 256)]),
                                start=(pi == 0 and j == 0),
                                stop=(pi == 2 and j == 1),
                                perf_mode=DR)

            def evict_k1(pk, tag):
                """k1 (pk cols [0,256)) -> SBUF bf16 [r, o] (col r*128+o)."""
                s = opool.tile([128, 256], bf16, tag=tag)
                nc.scalar.copy(
                    _apn(s[:], 0, [(1, 128), (128, 2)]),
                    _apn(pk[:], 0, [(2, 128), (1, 2)]))
                return s

            def recomb(pk, sk1, tag):
                """comps [j, r, o] bf16 (col j*256 + r*128 + o):
                Re = k2' + k1, Im = k3 + k1 in one broadcast dual op."""
                comps = opool.tile([128, 512], bf16, tag=tag)
                nc.vector.tensor_add(
                    _apn(comps[:], 0, [(256, 2), (1, 128), (128, 2)]),
                    _apn(pk[:], 256, [(256, 2), (2, 128), (1, 2)]),
                    _apn(sk1[:], 0, [(0, 2), (1, 128), (128, 2)]))
                return comps

            def bfly(c0, c1, h, stage, tail=False):
                """8 blades for o-half h; stage cols 1024h + l*128 + o_loc.
                Adds on DVE (2x packed bf16); subs on Pool mid-run, DVE on
                the tail."""
                base = 1024 * h
                o2 = [(1, 128)]
                sub2 = nc.vector.tensor_sub if tail else nc.gpsimd.tensor_sub
                nc.vector.tensor_add(                      # x0, x7
                    _apn(stage[:], base + 0 * 128, [(896, 2)] + o2),
                    _apn(c0[:], 0, [(256, 2)] + o2),
                    _apn(c1[:], 128, [(256, 2)] + o2))
                sub2(                                      # x4, x3
                    _apn(stage[:], base + 4 * 128, [(-128, 2)] + o2),
                    _apn(c0[:], 0, [(256, 2)] + o2),
                    _apn(c1[:], 128, [(256, 2)] + o2))
                nc.vector.tensor_add(                      # x1, x6
                    _apn(stage[:], base + 1 * 128, [(640, 2)] + o2),
                    _apn(c0[:], 128, [(256, 2)] + o2),
                    _apn(c1[:], 0, [(256, 2)] + o2))
                sub2(                                      # x5, x2
                    _apn(stage[:], base + 5 * 128, [(-384, 2)] + o2),
                    _apn(c0[:], 128, [(256, 2)] + o2),
                    _apn(c1[:], 0, [(256, 2)] + o2))

            def process_half(xs, h, ctag):
                """matmuls + k1 eviction + recombination for one (c, half)."""
                pk = pspool.tile([128, 1024], f32, tag=f"pk{ctag}")
                half_groups(xs, pk, h)
                sk1 = evict_k1(pk, f"sk{ctag}{h}")
                return recomb(pk, sk1, f"c{ctag}{h}")

            for bt in range(BT):
                last = bt == BT - 1
                if bt == 0:
                    x0_s, x1_s = x0_pre, x1_pre
                else:
                    x0_s = xpool.tile([128, 2, 3, 4, 128], fp8, tag="x0")
                    x1_s = xpool.tile([128, 2, 3, 4, 128], fp8, tag="x1")
                    nc.sync.dma_start(x0_s[:], xt0[bt])
                    nc.sync.dma_start(x1_s[:], xt1[bt])
                stage = opool.tile([128, OUTW], bf16, tag="stage")
                row = out[bt * 128:(bt + 1) * 128, :]
                if not last:
                    c0a = process_half(x0_s, 0, 0)
                    c0b = process_half(x0_s, 1, 0)
                    c1a = process_half(x1_s, 0, 1)
                    bfly(c0a, c1a, 0, stage)
                    c1b = process_half(x1_s, 1, 1)
                    bfly(c0b, c1b, 1, stage)
                    nc.sync.dma_start(row, stage[:])
                else:
                    # tail: c1 first, drain in column halves.
                    c1a = process_half(x1_s, 0, 1)
                    c1b = process_half(x1_s, 1, 1)
                    c0a = process_half(x0_s, 0, 0)
                    bfly(c0a, c1a, 0, stage, tail=True)
                    nc.sync.dma_start(row[:, 0:1024], stage[:, 0:1024])
                    c0b = process_half(x0_s, 1, 0)
                    bfly(c0b, c1b, 1, stage, tail=True)
                    nc.sync.dma_start(row[:, 1024:2048], stage[:, 1024:2048])
    nc.finalize()
    return nc


def _pauli_parts(v):
    """v[..., 8] -> c0, c1 of shape [..., 2(m), 2(reim)]: the c-th column
    (Re, Im) of phi(v) rows m."""
    c0 = np.empty(v.shape[:-1] + (2, 2), dtype=v.dtype)
    c1 = np.empty_like(c0)
    v0, v1, v2, v3, v4, v5, v6, v7 = (v[..., a] for a in range(8))
    c0[..., 0, 0] = v0 + v4   # Re A
    c0[..., 0, 1] = v3 + v7   # Im A
    c0[..., 1, 0] = v1 + v5   # Re C
    c0[..., 1, 1] = v6 + v2   # Im C
    c1[..., 0, 0] = v1 - v5   # Re B
    c1[..., 0, 1] = v6 - v2   # Im B
    c1[..., 1, 0] = v0 - v4   # Re D
    c1[..., 1, 1] = v7 - v3   # Im D
    return c0, c1


def _hi_lo(v):
    """f32 array -> (hi, lo) e4m3 planes with hi + lo ~= v."""
    hi = v.astype(E4)
    lo = (v - hi.astype(np.float32)).astype(E4)
    return hi, lo


def _prep_w(weight):
    """weight [COUT, CIN, 8] -> [2, 128, 6144] fp8 kpair blocks of the
    Gauss planes [Ar, -(Ar+Ai), Ai-Ar], each block [128 p, 2 t, 512 c]."""
    w = weight.astype(np.float32)
    cw0, cw1 = _pauli_parts(w)
    R = np.empty((CIN, 2, COUT, 2), np.float32)   # [(i,m),(o,r)]
    I = np.empty_like(R)
    for m, cm in ((0, cw0), (1, cw1)):
        for r in range(2):
            R[:, m, :, r] = 0.5 * cm[:, :, r, 0].T
            I[:, m, :, r] = 0.5 * cm[:, :, r, 1].T
    R = R.reshape(HK, HK) * SW
    I = I.reshape(HK, HK) * SW
    out = np.empty((2, 128, 6144), dtype=E4)
    for hl in (0, 1):
        blocks = []
        for M in (R, -(R + I), I - R):
            h, l = _hi_lo(M)
            P = (h if hl == 0 else l).astype(np.float32)
            for j in (0, 1):
                blk = P[256 * j:256 * j + 256].reshape(2, 128, HK)
                blocks.append(blk.transpose(1, 0, 2))   # [128, 2, 512]
        arr = np.stack(blocks, axis=1)    # [128, 6, 2, 512]
        out[hl] = arr.reshape(128, 6144).astype(E4)
    return out


def _prep_x(x):
    """x [B, CIN, 8] -> two per-core arrays [N_CORES, BT, 128, 3072] fp8
    (c = 0, 1): ops [xs, xi, xr] in [bt, p, hl, op, kt, b] layout."""
    xf = x.astype(np.float32)
    c0, c1 = _pauli_parts(xf)          # [B, CIN, m, reim]
    outs = []
    for arr in (c0, c1):
        kb = arr.transpose(3, 1, 2, 0).reshape(K, B) * SX   # [K, B]
        xr, xi = kb[0:HK], kb[HK:K]
        ops = np.stack([xr + xi, xi, xr], axis=0)   # [op, 512, B]
        a = ops.reshape(3, 4, 128, N_CORES, BT, 128)  # [op, kt, p, core, bt, b]
        a = np.ascontiguousarray(a.transpose(3, 4, 2, 0, 1, 5))
        hi, lo = _hi_lo(a)                       # [core, bt, p, op, kt, b]
        packed = np.stack([hi, lo], axis=3)      # [core, bt, p, hl, op, kt, b]
        outs.append(np.ascontiguousarray(
            packed.reshape(N_CORES, BT, 128, 3072)))
    return outs


def kernel(x, weight, bias, cayley):
    assert x.shape == (B, CIN, NB) and weight.shape == (COUT, CIN, NB)
    if "nc" not in _cached:
        _cached["nc"] = _build_nc()
    nc = _cached["nc"]

    xt0, xt1 = _prep_x(np.asarray(x))
    w8 = _prep_w(np.asarray(weight))
    in_maps = [{"xt0": xt0[c], "xt1": xt1[c], "w8": w8} for c in range(N_CORES)]
    res = run_bass_kernel_spmd(nc, in_maps, core_ids=list(range(N_CORES)))
    out = np.concatenate(
        [res.results[c]["out"].astype(np.float32) for c in range(N_CORES)],
        axis=0)
    # device cols: (h, l, o_loc) -> out[b, o, l] with o = h*128 + o_loc
    out = out.reshape(B, 2, NB, 128).transpose(0, 1, 3, 2).reshape(B, COUT, NB)
    out = out * (1.0 / (SX * SW))
    out = out + np.asarray(bias, np.float32)[None]
    return out.astype(np.float32)
